# revision 8
# baseline (speedup 1.0000x reference)
"""EnhancedMultiHeadAttention TRN2 kernel (8 NeuronCores).

Problem (hardcoded shapes): B=4, L=1024, HID=1024, H=16, DH=64, MAX_SEQ=1024.
  q/k/v = x @ W* + b*          (per-head split)
  S = q k^T / sqrt(64) + einsum('bhid,ijd->bhij', q, rel_emb[i-j+1023])
  attn = softmax(S); out = (attn @ v) @ Wo + bo
(The reference's extra renorm attn/(sum+1e-8) is a no-op after softmax.)

Sharding: core c -> batch b = c//2, head group g = c%2 (8 heads each).
Each core computes a partial output x[b]-block @ Wo-rows; host sums the two
partials per batch.

Rel-pos bias trick: qE[i, r'] = q_i . rel_emb[2046-r'] (one matmul against the
flipped table), then bias[i, j] = qE[i, 1023-i+j] is a diagonal re-read of qE
with row stride 1151 inside a [128, 1152] window - done as an SBUF->SBUF DMA
with a hand-built access pattern (flat-element semantics verified on HW),
issued on the GpSimd SWDGE ring (HWDGE rings carry transposes/loads).

The gathered bias is accumulated INTO the score PSUM by an identity matmul
(start=False on the same banks as the q.k matmuls), so no vector-engine add
is needed and exp reads the scores straight out of PSUM.

Key scheduling choices (v2, rebuilt from trace analysis of the 318us v1):
 - exp output is UNNORMALIZED; the 1/sum renorm is folded into the A.V PSUM
   evacuation (tensor_tensor multiply against a [128,1024] replicated
   reciprocal-row tile).  The recip row is built per pair:  PE-transpose of
   the exp accumulator columns [128,8] -> [8,128], DVE reciprocal, then a
   partition-replicating SWDGE DMA (contiguous 4KB source runs).
 - attention transposes are per-(tile, head) [128,1024] DMAs on the SP HWDGE
   ring issued right after each exp.  v1's whole-pair 2MB transposes starved
   the skew-gather DMAs at every pair boundary -> 5-9us PE stalls -> HAM
   re-throttled the PE clock to 1.2GHz for ~half the kernel.
 - A.V is split into (pair, i-column-half) units of 16 matmuls; unit (p,c0)
   drains 6/6/4 matmuls per step over steps (p,4..6), (p,c1) over (p+1,0..2).
   This removes the serial AV tail and keeps the PE queue dense.
 - the first 3 steps' qE evacuations go to ACT (nc.scalar.copy) so the skew
   gathers fire while the DVE is still draining V-projection evacuations;
   afterwards big chunks on DVE, the 256-wide tail chunk on ACT.

PSUM is statically partitioned:
  tag "sps" 2 x [128,1024] (4 banks): proj qt/kt accumulators, score tiles
  tag "qe"  3 x [128,512]  (3 banks): warmup, qE chunks, sums-transpose,
                                      out-proj
  tag "av"  1 x [128,512]  (1 bank):  V-proj accumulators, A.V accumulator

All GEMMs are bf16; scores/softmax fp32.  Head pairs emit their K=64 (qE/S)
and M=64 (AV) matmuls adjacently: the auto-derived tile_position row/col
groups (0,*) / (64,*) let the two heads' matmuls overlap in the array.
"""

import ml_dtypes
import numpy as np

import concourse.bass as bass
import concourse.mybir as mybir
import concourse.tile as tile
from concourse.bass_utils import run_bass_kernel_spmd

B, L, HID, H = 4, 1024, 1024, 16
DH = 64
HPC = 8          # heads per core
NPAIR = 4        # head pairs per core
NT = L // 128    # 8 i-tiles
F32 = mybir.dt.float32
BF16 = mybir.dt.bfloat16

_uid = [0]
FUSED_NORM = False  # False: baseline-style renorm multiply on the attn tile


def _split_multi_waits(nc):
    """Installed walrus accepts 1 sync-wait per instruction (2 for
    EventSemaphore); Tile's tail drain can carry more. Spill extras onto
    EventSemaphore wait-carriers inserted before the offender."""
    for f in nc.m.functions:
        for blk in f.blocks:
            insts = blk.instructions
            idx = 0
            while idx < len(insts):
                inst = insts[idx]
                si = inst.sync_info
                waits = list(si.on_wait) if si and si.on_wait else []
                cap = 2 if type(inst).__name__ == "InstEventSemaphore" else 1
                if len(waits) > cap:
                    si.on_wait = waits[:cap]
                    extra = waits[cap:]
                    carriers = []
                    for k in range(0, len(extra), 2):
                        _uid[0] += 1
                        nop = mybir.InstEventSemaphore(
                            name=f"wait_split_{_uid[0]}", ins=[], outs=[]
                        )
                        nop.engine = inst.engine
                        nop.sync_info = mybir.SyncInfo(
                            on_wait=extra[k:k + 2], on_update=[]
                        )
                        carriers.append(nop)
                    for c in reversed(carriers):
                        insts.insert(idx, c)
                    idx += len(carriers)
                idx += 1


def _ap_with(ap, dims, offset):
    """Return a copy of `ap` with raw [step,count] dims and element offset."""
    c = ap.copy()
    v = c.ap
    assert len(v) == len(dims), (v, dims)
    for i, d in enumerate(dims):
        v[i] = list(d)
    c.ap = v
    c.offset = offset
    return c


def _build_program():
    nc = bass.Bass()

    xT = nc.dram_tensor("xT", (HID, L), BF16, kind="ExternalInput")
    wq = nc.dram_tensor("wq", (HID, 512), BF16, kind="ExternalInput")
    wk = nc.dram_tensor("wk", (HID, 512), BF16, kind="ExternalInput")
    wv = nc.dram_tensor("wv", (HID, 512), BF16, kind="ExternalInput")
    wo = nc.dram_tensor("wo", (512, L), BF16, kind="ExternalInput")
    rt = nc.dram_tensor("rt", (128, 2048), BF16, kind="ExternalInput")
    ident = nc.dram_tensor("ident", (128, 128), BF16, kind="ExternalInput")
    ident32 = nc.dram_tensor("ident32", (128, 128), F32, kind="ExternalInput")
    bq = nc.dram_tensor("bq", (512,), F32, kind="ExternalInput")
    bk = nc.dram_tensor("bk", (512,), F32, kind="ExternalInput")
    bv = nc.dram_tensor("bv", (512,), F32, kind="ExternalInput")
    bo = nc.dram_tensor("bo", (L,), F32, kind="ExternalInput")
    out = nc.dram_tensor("out", (L, L), F32, kind="ExternalOutput")
    # per-pair staging rows for the reciprocal broadcast (SBUF sources can't
    # have 0-step partition dims, DRAM sources can)
    rrow = [nc.dram_tensor(f"rrow{p}", (2048,), F32, kind="Internal")
            for p in range(NPAIR)]

    with tile.TileContext(nc) as tc:
        with tc.tile_pool(name="weights", bufs=1) as wpool, \
             tc.tile_pool(name="proj", bufs=1) as projpool, \
             tc.tile_pool(name="ps", bufs=1, space="PSUM") as ps:

            # ---- resident small tensors ----
            rt_sb = wpool.tile([128, 2048], BF16)
            nc.sync.dma_start(rt_sb[:], rt[:])
            id_sb = wpool.tile([128, 128], BF16)
            nc.sync.dma_start(id_sb[:], ident[:])
            id32_sb = wpool.tile([128, 128], F32)
            nc.sync.dma_start(id32_sb[:], ident32[:])
            bq_sb = wpool.tile([128, 4], F32)
            nc.sync.dma_start(bq_sb[:], bq[:].rearrange("(t p) -> p t", p=128))
            bk_sb = wpool.tile([128, 4], F32)
            nc.sync.dma_start(bk_sb[:], bk[:].rearrange("(t p) -> p t", p=128))
            # bv replicated across partitions: [1,512] read with partition step 0
            bv_sb = wpool.tile([128, 512], F32)
            nc.sync.dma_start(bv_sb[:], _ap_with(bv[None, :], [[0, 128], [1, 512]], 0))
            bo_sb = wpool.tile([128, 1024], F32)
            nc.sync.dma_start(bo_sb[:], _ap_with(bo[None, :], [[0, 128], [1, 1024]], 0))

            # HAM warm-up: ~4us of junk matmuls on the first-loaded tile so
            # the PE clock is at 8/8 before the projection phase starts
            wu_ps = ps.tile([128, 512], F32, tag="qe", bufs=3, name="wu_ps")
            for wi in range(24):
                nc.tensor.matmul(wu_ps[:], rt_sb[:, 0:128], rt_sb[:, 0:512],
                                 start=(wi == 0), stop=(wi == 23))

            # ---- projections: QT/KT [qdim part, seq], V [seq part, vdim] ----
            qt_sb = [projpool.tile([128, L], BF16, name=f"qt{m}") for m in range(4)]
            kt_sb = [projpool.tile([128, L], BF16, name=f"kt{m}") for m in range(4)]
            v_sb = [projpool.tile([128, 512], BF16, name=f"v{t}") for t in range(NT)]

            with tc.tile_pool(name="p1", bufs=1) as p1:
                # batched loads: one DMA per tensor (32 small DMAs would eat
                # ~13us of descriptor-gen on the SP ring before proj can run)
                xk = p1.tile([128, 8, L], BF16, name="xk")
                wqk = p1.tile([128, 8, 512], BF16, name="wqk")
                wkk = p1.tile([128, 8, 512], BF16, name="wkk")
                wvk = p1.tile([128, 8, 512], BF16, name="wvk")
                xT_r = xT[:].rearrange("(k p) c -> p k c", p=128)
                wq_r = wq[:].rearrange("(k p) c -> p k c", p=128)
                wk_r = wk[:].rearrange("(k p) c -> p k c", p=128)
                wv_r = wv[:].rearrange("(k p) c -> p k c", p=128)
                for lo, hi in ((0, 2), (2, 8)):
                    nc.sync.dma_start(xk[:, lo:hi, :], xT_r[:, lo:hi, :])
                    nc.sync.dma_start(wqk[:, lo:hi, :], wq_r[:, lo:hi, :])
                    nc.sync.dma_start(wkk[:, lo:hi, :], wk_r[:, lo:hi, :])
                    nc.sync.dma_start(wvk[:, lo:hi, :], wv_r[:, lo:hi, :])

                for m in range(4):
                    msl = slice(m * 128, (m + 1) * 128)
                    qt_ps = ps.tile([128, 1024], F32, tag="sps", bufs=2,
                                    name=f"qtp{m}")
                    kt_ps = ps.tile([128, 1024], F32, tag="sps", bufs=2,
                                    name=f"ktp{m}")
                    # stationary wqk[:,k,msl] reused for both c-halves
                    for k in range(8):
                        for c in range(2):
                            csl = slice(c * 512, (c + 1) * 512)
                            nc.tensor.matmul(qt_ps[:, csl], wqk[:, k, msl],
                                             xk[:, k, csl],
                                             start=(k == 0), stop=(k == 7))
                    for k in range(8):
                        for c in range(2):
                            csl = slice(c * 512, (c + 1) * 512)
                            nc.tensor.matmul(kt_ps[:, csl], wkk[:, k, msl],
                                             xk[:, k, csl],
                                             start=(k == 0), stop=(k == 7))
                    nc.any.tensor_scalar_add(qt_sb[m][:], qt_ps[:],
                                             bq_sb[:, m:m + 1])
                    nc.any.tensor_scalar_add(kt_sb[m][:], kt_ps[:],
                                             bk_sb[:, m:m + 1])
                for t in range(NT):
                    tsl = slice(t * 128, (t + 1) * 128)
                    v_ps = ps.tile([128, 512], F32, tag="av", bufs=1,
                                   name=f"vp{t}")
                    for k in range(8):
                        nc.tensor.matmul(v_ps[:], xk[:, k, tsl], wvk[:, k, :],
                                         start=(k == 0), stop=(k == 7))
                    nc.vector.tensor_tensor(
                        v_sb[t][:], v_ps[:], bv_sb[:], mybir.AluOpType.add)

            wo_sb = wpool.tile([128, 4, L], BF16, name="wo_sb")
            nc.sync.dma_start(wo_sb[:], wo[:].rearrange("(m p) c -> p m c", p=128))

            # ---- attention ----
            work = tc.alloc_tile_pool(name="work", bufs=3)
            apool = tc.alloc_tile_pool(name="attn", bufs=2)
            opool = tc.alloc_tile_pool(name="outp", bufs=3)
            ctxT_sb = [None] * NPAIR

            PFD = 3  # bias prefetch distance, in global (p,t) steps
            seq = [(p, t) for p in range(NPAIR) for t in range(NT)]
            NSTEP = len(seq)

            sums_all = {}    # p -> [2 x [128,8] f32]
            aT_all = {}      # p -> [2 x [128,8,8,128] bf16]
            bias_tiles = {}  # gi -> [2 x [128,1024] bf16]
            rrep_all = {}    # p -> [128,1024] f32 replicated recip rows
            av_ps_all = {}   # (p,c) -> psum tile
            av_queue = []    # pending AV matmul thunks

            def emit_qe(gi):
                """qE for both heads of pair p, i-tile t.  Head h's matmuls
                use qt/rt partitions [64h, 64h+64) -> auto tile_position
                (64h, 0); same-chunk matmuls of the two heads are adjacent in
                the PE queue so their row groups overlap in the array."""
                p, t = seq[gi]
                qt_p = qt_sb[p]
                w0 = 896 - 128 * t
                isl = slice(t * 128, (t + 1) * 128)
                qe_t = [work.tile([128, 1152], BF16, tag=f"qe{h}", bufs=5,
                                  name=f"qe_sb{h}") for h in range(2)]
                chunks = ((0, 512), (512, 384), (896, 256))
                qe_ps = {}
                for ci, (c0, cw) in enumerate(chunks):
                    for h in range(2):
                        hs = slice(64 * h, 64 * h + 64)
                        qe_ps[ci, h] = ps.tile([128, 512], F32, tag="qe",
                                               bufs=3, name=f"qe{p}{t}{h}{ci}")
                        nc.tensor.matmul(
                            qe_ps[ci, h][:, :cw], qt_p[hs, isl],
                            rt_sb[hs, w0 + c0:w0 + c0 + cw],
                            start=True, stop=True)
                for ci, (c0, cw) in enumerate(chunks):
                    for h in range(2):
                        # PSUM evacuation: first steps all on ACT (DVE is
                        # still draining V-proj evacs and the gathers must
                        # fire NOW); steady state big chunks on DVE, the
                        # 256-wide tail chunks on ACT (keeps both balanced)
                        if gi >= PFD and ci < 2:
                            nc.vector.tensor_copy(qe_t[h][:, c0:c0 + cw],
                                                  qe_ps[ci, h][:, :cw])
                        else:
                            nc.scalar.copy(qe_t[h][:, c0:c0 + cw],
                                           qe_ps[ci, h][:, :cw])
                # skew gather: bias[q, j] = qe_sb[q, 127 - q + j]
                bias_t = [work.tile([128, L], BF16, tag=f"bias{h}", bufs=6,
                                    name=f"bias_sb{h}") for h in range(2)]
                for h in range(2):
                    nc.gpsimd.dma_start(
                        bias_t[h][:],
                        _ap_with(qe_t[h][:, 0:1024],
                                 [[1151, 128], [1, 1024]], 127))
                return bias_t

            def emit_s(p, t, bias_t):
                """scores for both heads of pair p, i-tile t: q.k matmuls
                (start=True) then identity matmuls accumulating the gathered
                bias into the same PSUM banks (stop=True); exp reads the
                2-bank score tile straight from PSUM, leaves the row sums in
                the accumulator tile, and the attn tile goes straight to a
                per-tile transpose DMA (renorm is folded into the AV evac)."""
                qt_p = qt_sb[p]
                kt_p = kt_sb[p]
                isl = slice(t * 128, (t + 1) * 128)
                s_ps = [ps.tile([128, 1024], F32, tag="sps", bufs=2,
                                name=f"s_ps{h}") for h in range(2)]
                for c in range(2):
                    csl = slice(c * 512, (c + 1) * 512)
                    for h in range(2):
                        hs = slice(64 * h, 64 * h + 64)
                        nc.tensor.matmul(
                            s_ps[h][:, csl], qt_p[hs, isl], kt_p[hs, csl],
                            start=True, stop=False)
                for c in range(2):
                    csl = slice(c * 512, (c + 1) * 512)
                    for h in range(2):
                        nc.tensor.matmul(
                            s_ps[h][:, csl], id_sb[:], bias_t[h][:, csl],
                            start=False, stop=True)
                for h in range(2):
                    at = apool.tile([128, L], BF16, tag=f"attn{h}", bufs=4,
                                    name=f"attn{h}")
                    nc.scalar.activation(
                        at[:], s_ps[h][:],
                        mybir.ActivationFunctionType.Exp,
                        accum_out=sums_all[p][h][:, t:t + 1])
                    if not FUSED_NORM:
                        recip = work.tile([128, 1], F32, tag=f"recip{h}",
                                          bufs=4, name=f"recip{h}")
                        nc.vector.reciprocal(recip[:],
                                             sums_all[p][h][:, t:t + 1])
                        nc.vector.tensor_scalar_mul(at[:], at[:], recip[:])
                    nc.sync.dma_start(aT_all[p][h][:, t, :, :], at[:],
                                      transpose=True)

            def start_av(p, c):
                """Allocate the AV accumulator and build the 16 matmul thunks
                for unit (p, c) = i-tiles [4c, 4c+4) x all j; drained a few
                per step so the PE queue never bunches."""
                av_ps = ps.tile([128, 512], F32, tag="av", bufs=1,
                                name=f"avp{p}{c}")
                av_ps_all[p, c] = av_ps
                thunks = []
                for jt in range(NT):
                    for h in range(2):
                        def mk(jt=jt, h=h, av_ps=av_ps, p=p, c=c):
                            nc.tensor.matmul(
                                av_ps[64 * h:64 * h + 64, :],
                                v_sb[jt][:, 64 * (2 * p + h):
                                          64 * (2 * p + h) + 64],
                                aT_all[p][h][:, 4 * c:4 * (c + 1), jt, :],
                                start=(jt == 0), stop=(jt == NT - 1))
                        thunks.append(mk)
                return thunks

            def finish_av(p, c):
                """Fused evacuation: ctxT = av_ps * replicated 1/rowsum."""
                csl = slice(c * 512, (c + 1) * 512)
                if FUSED_NORM:
                    nc.vector.tensor_tensor(
                        ctxT_sb[p][:, csl], av_ps_all.pop((p, c))[:],
                        rrep_all[p][:, csl], mybir.AluOpType.mult)
                elif c == 0:
                    nc.vector.tensor_copy(ctxT_sb[p][:, csl],
                                          av_ps_all.pop((p, c))[:])
                else:
                    nc.scalar.copy(ctxT_sb[p][:, csl],
                                   av_ps_all.pop((p, c))[:])

            def pair_end_chain(p):
                """After pair p's last exp: build the replicated reciprocal
                row tile, evacuate AV(p,c0), and queue AV(p,c1)."""
                if FUSED_NORM:
                    tr_ps = ps.tile([128, 512], F32, tag="qe", bufs=3,
                                    name=f"trp{p}")
                    for h in range(2):
                        nc.tensor.transpose(tr_ps[0:8, 128 * h:128 * h + 128],
                                            sums_all[p][h][:, 0:8],
                                            id32_sb[:])
                    rcp = work.tile([128, 2, 128], F32, tag="rc", bufs=2,
                                    name=f"rc{p}")
                    for h in range(2):
                        nc.vector.reciprocal(rcp[0:8, h, :],
                                             tr_ps[0:8,
                                                   128 * h:128 * h + 128])
                    rrep = work.tile([128, 1024], F32, tag="rrep", bufs=2,
                                     name=f"rrep{p}")
                    rrep_all[p] = rrep
                    # straighten the two [8,128] recip blocks into one DRAM
                    # row (h-major), then broadcast-load it with a 0-step rep
                    # dim: head h's row lands on partitions [64h, 64h+64)
                    nc.gpsimd.dma_start(
                        rrow[p][:],
                        _ap_with(rcp[:], [[128, 2], [256, 8], [1, 128]], 0))
                    nc.gpsimd.dma_start(
                        rrep[:],
                        _ap_with(rrow[p][None, None, :],
                                 [[1024, 2], [0, 64], [1, 1024]], 0))
                ctxT_sb[p] = projpool.tile([128, L], BF16, name=f"ctxT{p}")
                finish_av(p, 0)
                sums_all.pop(p)

            # global software pipeline over all (p, t) steps: the qE/skew
            # chain runs PFD steps ahead and crosses pair boundaries, so the
            # first score-blocks of pair p+1 never wait on a cold skew chain.
            for gi in range(NSTEP + PFD):
                if gi < NSTEP:
                    bias_tiles[gi] = emit_qe(gi)
                if gi < PFD:
                    continue
                si = gi - PFD
                p, t = seq[si]
                if t == 0:
                    # allocate here, NOT in the prefetch branch: at this
                    # point every instruction of pair p-1 (exp tail,
                    # transposes) has been emitted, so ring reuse sees
                    # all prior references and can't clobber live tiles
                    sums_all[p] = [
                        work.tile([128, 8], F32, tag=f"sums{h}", bufs=2,
                                  name=f"sums{p}_{h}") for h in range(2)]
                    aT_all[p] = [
                        apool.tile([128, NT, NT, 128], BF16, tag=f"aT{h}",
                                   bufs=2, name=f"aT{p}_{h}")
                        for h in range(2)]
                    if p > 0:
                        pair_end_chain(p - 1)
                        av_queue.extend(start_av(p - 1, 1))
                if t == 3 and p > 0:
                    finish_av(p - 1, 1)
                    aT_all.pop(p - 1)
                for _ in range(min(6, len(av_queue))):
                    av_queue.pop(0)()
                emit_s(p, t, bias_tiles.pop(si))
                if t == 3:
                    av_queue.extend(start_av(p, 0))

            # tail: last pair's recip chain, AV(3,c1), out-projection
            pair_end_chain(NPAIR - 1)
            av_queue.extend(start_av(NPAIR - 1, 1))
            for th in av_queue:
                th()
            av_queue.clear()
            finish_av(NPAIR - 1, 1)
            aT_all.pop(NPAIR - 1)

            # ---- output projection (transpose-mode: ctx[i,hd] @ Wo[hd,o]) ----
            for t in range(NT):
                isl = slice(t * 128, (t + 1) * 128)
                for c in range(2):
                    o_ps = ps.tile([128, 512], F32, tag="qe", bufs=3,
                                   name=f"o{t}{c}")
                    for m in range(4):
                        nc.tensor.matmul(
                            o_ps[:], ctxT_sb[m][:, isl],
                            wo_sb[:, m, c * 512:(c + 1) * 512],
                            start=(m == 0), stop=(m == 3))
                    o_sb = opool.tile([128, 512], F32, tag="osb")
                    nc.vector.tensor_tensor(
                        o_sb[:], o_ps[:], bo_sb[:, c * 512:(c + 1) * 512],
                        mybir.AluOpType.add)
                    nc.sync.dma_start(out[isl, c * 512:(c + 1) * 512], o_sb[:])
            opool.release()
            apool.release()
            work.release()

    _split_multi_waits(nc)
    return nc


_cached = {}


def _get_program():
    if "nc" not in _cached:
        _cached["nc"] = _build_program()
    return _cached["nc"]


def kernel(x, Wq, bq, Wk, bk, Wv, bv, Wo, bo, rel_emb, _timing=None):
    x = np.asarray(x, np.float32)
    Wq = np.asarray(Wq, np.float32)
    Wk = np.asarray(Wk, np.float32)
    Wv = np.asarray(Wv, np.float32)
    Wo = np.asarray(Wo, np.float32)
    bq_ = np.asarray(bq, np.float32)
    bk_ = np.asarray(bk, np.float32)
    bv_ = np.asarray(bv, np.float32)
    bo_ = np.asarray(bo, np.float32)
    rel = np.asarray(rel_emb, np.float32)

    # flipped rel table, transposed, duplicated on both 64-partition halves,
    # padded to 2048 cols
    rt_half = rel[::-1, :].T  # [64, 2047]
    rt_np = np.zeros((128, 2048), ml_dtypes.bfloat16)
    rt_np[0:64, 0:2047] = rt_half.astype(ml_dtypes.bfloat16)
    rt_np[64:128, 0:2047] = rt_half.astype(ml_dtypes.bfloat16)

    bf = ml_dtypes.bfloat16
    id_np = np.eye(128, dtype=bf)
    id32_np = np.eye(128, dtype=np.float32)
    in_maps = []
    for core in range(8):
        b, g = divmod(core, 2)
        cols = slice(g * 512, (g + 1) * 512)
        in_maps.append({
            "xT": np.ascontiguousarray(x[b].T).astype(bf),
            "wq": np.ascontiguousarray(Wq[:, cols]).astype(bf),
            "wk": (np.ascontiguousarray(Wk[:, cols]) / 8.0).astype(bf),
            "wv": np.ascontiguousarray(Wv[:, cols]).astype(bf),
            "wo": np.ascontiguousarray(Wo[cols, :]).astype(bf),
            "rt": rt_np,
            "ident": id_np,
            "ident32": id32_np,
            "bq": np.ascontiguousarray(bq_[cols]),
            "bk": np.ascontiguousarray(bk_[cols]) / 8.0,
            "bv": np.ascontiguousarray(bv_[cols]),
            "bo": bo_ if g == 0 else np.zeros_like(bo_),
        })

    nc = _get_program()
    kwargs = {}
    if _timing is not None:
        kwargs = dict(trace=True, trace_cores=list(range(8)))
    r = run_bass_kernel_spmd(nc, in_maps, core_ids=list(range(8)), **kwargs)
    if _timing is not None:
        _timing["exec_time_ns"] = r.exec_time_ns
        _timing["mean_exec_time_ns"] = r.mean_exec_time_ns
        _timing["trace"] = r.instructions_and_trace
    outs = [r.results[c]["out"] for c in range(8)]
    return np.stack([outs[2 * b] + outs[2 * b + 1] for b in range(B)], axis=0)
